# revision 2
# baseline (speedup 1.0000x reference)
"""Dilated local attention (ks=3, dil=2) on Trainium2, 8-core SPMD — v6.

Per position n, unit u (32-chan head): logits l_i = SCALE * <q_u[:,n], k_u[:,n+2i-2]>
(zero-padded edges), e_i = exp(l_i), denom = e_0+e_1+e_2+6,
out_u[:, n] = sum_i e_i * v_u[:, n+2i-2] / denom.

Host: f16 input casts + final transpose and softmax divide (device outputs are
UNNORMALIZED bf16 sums + per-unit exp rows; bf16 needed — logits reach ±18).

Per 512-col slab (16 slabs), software-pipelined across engines:
  prods   DVE  3 tensor_muls Pq_i = qc * kc(shift 2i), JIT 1024-col pieces (2x)
  logits  PE   3 matmuls lhsT=cl_bc_i [128,128] (SCALE iff same unit) ->
               psLb[:, 512i:+512] f32: logits BROADCAST to the unit's 32 rows
               (output rows are free on PE; each slice stays in one PSUM bank)
  exp     ACT  ONE op: E1bc [128,1536] bf16 = Exp(psLb 3-bank supertile);
               only Exp+Copy on ACT => single act-table load total
  vmult   tap0+part(tap1) DVE, rest(tap1)+tap2 Pool: T_i = vc(shift) * E1bc_i
  idsum   PE   3 accumulating identity matmuls -> psT f32 (tap-sum on PE)
  evac    ACT/DVE column-split copy psT -> ob pair tile bf16; DMA per pair
  S       DMA  E1bc rows {0,32,64,96} (per-pair) -> s_d; host sums taps + 6
  tail    last 2 slabs take a shallow all-DVE adds path to cut the drain
"""

import numpy as np

import concourse.bass as bass
import concourse.bacc as bacc
import concourse.mybir as mybir
import concourse.tile as tile
from concourse.bass_utils import run_bass_kernel_spmd
from concourse.mybir import AluOpType as A

B, D, N = 4, 256, 8192
HD = 32
SCALE = float(HD) ** -0.5
NCORES = 8
P = 128
UN = P // HD          # units per core (4)
f32 = mybir.dt.float32
f16 = mybir.dt.float16
bf16 = mybir.dt.bfloat16
AF = mybir.ActivationFunctionType
MUL = A.mult
ADD = A.add


def _consts():
    # cl_bc[p, 3*128]: tap-i slice [:, 128i:128(i+1)]: SCALE iff same unit
    cl_bc = np.zeros((P, 3 * P), np.float16)
    for i in range(3):
        for p in range(P):
            u = p // HD
            cl_bc[p, i * P + u * HD : i * P + (u + 1) * HD] = SCALE
    ident = np.eye(P, dtype=np.float16)
    return cl_bc, ident


def build_kernel(nc, n=N, cs=512, ldc=2048, reps=1, pool_add=True, pf=2,
                 acols=224, pvm1=192):
    """cs: slab width (psum bank f32); ldc: DMA/product chunk width.
    acols: psT-evac columns on ACT (rest DVE); pvm1: vmult tap-1 columns on
    Pool (rest DVE; tap 2 fully on Pool)."""
    assert n % ldc == 0 and ldc % cs == 0
    nslab = n // cs
    spc = ldc // cs          # slabs per chunk
    nchunk = n // ldc

    q_d = nc.declare_dram_parameter("q", [P, n], f16, isOutput=False)
    k_d = nc.declare_dram_parameter("k", [P, n], f16, isOutput=False)
    v_d = nc.declare_dram_parameter("v", [P, n], f16, isOutput=False)
    clbc_d = nc.declare_dram_parameter("cl_bc", [P, 3 * P], f16, isOutput=False)
    ident_d = nc.declare_dram_parameter("ident", [P, P], f16, isOutput=False)
    out_d = nc.declare_dram_parameter("out", [P, n], bf16, isOutput=True)
    s_d = nc.declare_dram_parameter("s", [UN, 3 * n], bf16, isOutput=True)

    with tile.TileContext(nc) as tc:
        with (
            tc.tile_pool(name="const", bufs=1) as const_pool,
            tc.tile_pool(name="big", bufs=1) as big_pool,
            tc.tile_pool(name="pq", bufs=2) as pq_pool,
            tc.tile_pool(name="e1", bufs=4) as e1_pool,
            tc.tile_pool(name="tt", bufs=5) as tt_pool,
            tc.tile_pool(name="ob", bufs=3) as ob_pool,
            tc.tile_pool(name="psLb", bufs=2, space="PSUM") as psLb_pool,
            tc.tile_pool(name="psT", bufs=2, space="PSUM") as psT_pool,
        ):
            qc = big_pool.tile([P, n], f16)
            kc = big_pool.tile([P, n + 4], f16)
            vc = big_pool.tile([P, n + 4], f16)
            clbc_t = const_pool.tile([P, 3 * P], f16)
            ident_t = const_pool.tile([P, P], f16)
            nc.gpsimd.memset(kc[:, 0:2], 0.0)
            nc.gpsimd.memset(kc[:, n + 2 : n + 4], 0.0)
            nc.gpsimd.memset(vc[:, 0:2], 0.0)
            nc.gpsimd.memset(vc[:, n + 2 : n + 4], 0.0)

            state = {}

            def dma_in(c, w=None):
                w = w or ldc
                for off in range(c * ldc, (c + 1) * ldc, w):
                    nc.sync.dma_start(out=qc[:, off : off + w],
                                      in_=q_d[:, off : off + w])
                    nc.sync.dma_start(out=kc[:, 2 + off : 2 + off + w],
                                      in_=k_d[:, off : off + w])
                    nc.sync.dma_start(out=vc[:, 2 + off : 2 + off + w],
                                      in_=v_d[:, off : off + w])

            def prods_piece(p):
                # piece p covers slabs 2p, 2p+1 (1024 cols); pq tiles per chunk
                # kc col (n + 2i) holds k[n + 2i - 2]
                w = 2 * cs
                n0 = p * w
                if n0 >= n:
                    return
                c = n0 // ldc
                if n0 % ldc == 0:
                    state[("pq", c)] = [
                        pq_pool.tile([P, ldc], f16, name=f"pq{i}") for i in range(3)]
                pq = state[("pq", c)]
                off = n0 % ldc
                for i in range(3):
                    nc.vector.tensor_mul(
                        pq[i][:, off : off + w], qc[:, n0 : n0 + w],
                        kc[:, n0 + 2 * i : n0 + 2 * i + w])

            def logits(s):
                pq = state[("pq", s // spc)]
                off = (s % spc) * cs
                psLb = psLb_pool.tile([P, 3 * cs], f32, name="psLb")
                for i in range(3):
                    nc.tensor.matmul(
                        psLb[:, cs * i : cs * (i + 1)],
                        clbc_t[:, P * i : P * (i + 1)],
                        pq[i][:, off : off + cs],
                        start=True, stop=True)
                state[("L", s)] = psLb

            def exp(s):
                # pairs of slabs share one e1 tile so S-DMA goes per pair
                psLb = state.pop(("L", s))
                if s % 2 == 0:
                    e1d = e1_pool.tile([P, 6 * cs], bf16, name="e1")
                    state[("Ed", s // 2)] = e1d
                else:
                    e1d = state[("Ed", s // 2)]
                half = e1d[:, (s % 2) * 3 * cs : (s % 2 + 1) * 3 * cs]
                nc.scalar.activation(half, psLb[:], AF.Exp)
                state[("E", s)] = half

            def vmult(s, tail=False):
                # tap 0 DVE; tap 1 split DVE/Pool at pvm1; tap 2 Pool.
                # tail slabs: all-DVE + immediate adds + per-slab out DMA
                # (shallow drain, no idsum/evac).
                e1 = state[("E", s)]
                n0 = s * cs
                ts = [tt_pool.tile([P, cs], bf16, name=f"t{i}") for i in range(3)]
                nc.vector.tensor_mul(ts[0][:], vc[:, n0 : n0 + cs],
                                     e1[:, 0:cs])
                if tail:
                    ob = ob_pool.tile([P, cs], bf16, name="ob")
                    nc.vector.tensor_mul(ts[1][:], vc[:, n0 + 2 : n0 + 2 + cs],
                                         e1[:, cs : 2 * cs])
                    nc.gpsimd.tensor_mul(ts[2][:], vc[:, n0 + 4 : n0 + 4 + cs],
                                         e1[:, 2 * cs : 3 * cs])
                    nc.vector.tensor_add(ts[0][:], ts[0][:], ts[1][:])
                    nc.vector.tensor_add(ob[:], ts[0][:], ts[2][:])
                    nc.sync.dma_start(out=out_d[:, n0 : n0 + cs], in_=ob[:])
                    return
                if pvm1 < cs:
                    nc.vector.tensor_mul(
                        ts[1][:, 0 : cs - pvm1],
                        vc[:, n0 + 2 : n0 + 2 + cs - pvm1],
                        e1[:, cs : 2 * cs - pvm1])
                if pvm1 > 0:
                    nc.gpsimd.tensor_mul(
                        ts[1][:, cs - pvm1 : cs],
                        vc[:, n0 + 2 + cs - pvm1 : n0 + 2 + cs],
                        e1[:, 2 * cs - pvm1 : 2 * cs])
                nc.gpsimd.tensor_mul(
                    ts[2][:], vc[:, n0 + 4 : n0 + 4 + cs], e1[:, 2 * cs : 3 * cs])
                state[("T", s)] = ts

            def idsum(s):
                # tap-sum via accumulating identity matmuls on PE
                ts = state.pop(("T", s))
                psT = psT_pool.tile([P, cs], f32, name="psT")
                for i in range(3):
                    nc.tensor.matmul(psT[:], ident_t[:], ts[i][:],
                                     start=(i == 0), stop=(i == 2))
                state[("PT", s)] = psT

            def add2(s):
                # evacuate psT f32 -> ob pair tile bf16, split ACT / DVE;
                # DMA once per slab pair (fewer HWDGE slots)
                psT = state.pop(("PT", s))
                if s % 2 == 0:
                    obp = ob_pool.tile([P, 2 * cs], bf16, name="obp")
                    state[("OB", s // 2)] = obp
                else:
                    obp = state[("OB", s // 2)]
                o0 = (s % 2) * cs
                if acols > 0:
                    nc.scalar.copy(obp[:, o0 : o0 + acols], psT[:, 0:acols])
                if acols < cs:
                    nc.vector.tensor_copy(obp[:, o0 + acols : o0 + cs],
                                          psT[:, acols:cs])
                if s % 2 == 1:
                    state.pop(("OB", s // 2))
                    nc.sync.dma_start(
                        out=out_d[:, (s - 1) * cs : (s + 1) * cs], in_=obp[:])

            def sdma(s):
                state.pop(("E", s))
                if s % 2 == 1:
                    e1d = state.pop(("Ed", s // 2))
                    nc.sync.dma_start(
                        out=s_d[:, (s - 1) * 3 * cs : (s + 1) * 3 * cs],
                        in_=e1d[0 : P : HD, :])

            def body():
                state.clear()
                # inputs streamed in consumption order as 1024-col pieces for
                # chunks 0-1 (q/k before v; consts interleaved) so compute
                # starts ~2.5us in and never starves during fill.
                h = 1024
                nc.sync.dma_start(out=qc[:, 0:h], in_=q_d[:, 0:h])
                nc.sync.dma_start(out=kc[:, 2 : 2 + h + 4], in_=k_d[:, 0 : h + 4])
                nc.sync.dma_start(out=clbc_t[:], in_=clbc_d[:])
                nc.sync.dma_start(out=vc[:, 2 : 2 + h + 4], in_=v_d[:, 0 : h + 4])
                nc.sync.dma_start(out=ident_t[:], in_=ident_d[:])
                for off in range(h, min(2 * ldc, n), h):
                    nc.sync.dma_start(out=qc[:, off : off + h],
                                      in_=q_d[:, off : off + h])
                    nc.sync.dma_start(out=kc[:, 2 + off + 4 : 2 + off + h + 4],
                                      in_=k_d[:, off + 4 : off + h + 4])
                    nc.sync.dma_start(out=vc[:, 2 + off + 4 : 2 + off + h + 4],
                                      in_=v_d[:, off + 4 : off + h + 4])
                prods_piece(0)
                prods_piece(1)
                for r in range(nslab + 9):
                    c = r // spc + pf
                    if r % spc == 0 and c < nchunk:
                        dma_in(c)
                    if r % 2 == 0:
                        prods_piece(r // 2 + 2)
                    if r < nslab:
                        logits(r)
                    # idsum 3 rounds behind vmult so its inputs are long done
                    if 0 <= r - 5 < nslab - 2:
                        idsum(r - 5)
                    if 0 <= r - 1 < nslab:
                        exp(r - 1)
                    if 0 <= r - 6 < nslab - 2:
                        add2(r - 6)
                    if 0 <= r - 2 < nslab:
                        vmult(r - 2, tail=(r - 2 >= nslab - 2))
                    if 0 <= r - 3 < nslab:
                        sdma(r - 3)


            if reps == 1:
                body()
            else:
                with tc.For_i(0, reps, 1):
                    body()
    return nc


_cache = {}


def _get_nc():
    if "nc" not in _cache:
        nc = bacc.Bacc(None, target_bir_lowering=False, debug=False)
        build_kernel(nc)
        nc.compile()
        _cache["nc"] = nc
    return _cache["nc"]


def make_in_maps(q, k, v):
    cl_bc, ident = _consts()
    q16 = np.asarray(q, np.float16)
    k16 = np.asarray(k, np.float16)
    v16 = np.asarray(v, np.float16)
    in_maps = []
    for ci in range(NCORES):
        b = ci // 2
        c0 = (ci % 2) * P
        in_maps.append({
            "q": np.ascontiguousarray(q16[b, c0 : c0 + P, :]),
            "k": np.ascontiguousarray(k16[b, c0 : c0 + P, :]),
            "v": np.ascontiguousarray(v16[b, c0 : c0 + P, :]),
            "cl_bc": cl_bc,
            "ident": ident,
        })
    return in_maps


def run_sharded(q, k, v, **spmd_kwargs):
    q = np.asarray(q)
    k = np.asarray(k)
    v = np.asarray(v)
    assert q.shape == (B, D, N)
    nc = _get_nc()
    in_maps = make_in_maps(q, k, v)
    res = run_bass_kernel_spmd(nc, in_maps, list(range(NCORES)), **spmd_kwargs)
    out = np.empty((B, N, D), np.float32)
    cs = 512
    for ci, r in enumerate(res.results):
        b = ci // 2
        c0 = (ci % 2) * P
        t = r["out"].astype(np.float32)          # [128, n]
        # s: [4, 3n] f16 — per slab: 3 taps x cs cols of per-unit exp rows
        s = r["s"].astype(np.float32).reshape(UN, N // cs, 3, cs)
        denom = s.sum(axis=2).reshape(UN, N) + 6.0
        t /= np.repeat(denom, HD, axis=0)
        out[b, :, c0 : c0 + P] = t.T
    return out, res


def kernel(q, k, v):
    return run_sharded(q, k, v)[0]
